# revision 4
# baseline (speedup 1.0000x reference)
"""CrossAttention Trainium2 kernel (8 NeuronCores, Bass/Tile).

Problem: B=4, Nq=Nk=2048, DIM=1024, HEADS=16, HEAD_DIM=64, fp32.
  q = query @ Wq + bq ; k = key @ Wk + bk ; v = value @ Wv + bv
  attn = softmax(q k^T / 8) ; x = attn v ; out = x @ Wo + bo

Sharding: 8 cores = 4 batches x 2 head-groups (8 heads, 512 channels each).
Each core computes y_partial[b] = (attn-out restricted to its 512 channels)
@ Wo_rows; host sums the two partials per batch and adds bo.

v2 design (ACT-bound software pipeline):
  - The softmax exp is the hard floor: 8 heads x 2048^2 scores = 33.5M
    ACT-lane-cycles ~ 270us busy. Everything else hides under it:
    * All projections + outproj interleave with the attention loop
      (per-pair: K/Q proj of pair p+1 emitted inside pair p's attention).
    * ACT does EXP only; every copy/bias-add/normalize is DVE.
  - QK is row-tiled: the two heads of a pair contract K=64 each on row
    groups (0,0)/(64,0) concurrently -> no zero-padding waste.
  - AV keeps the [v_even|ones|v_odd] packing: the ones columns produce the
    softmax denominator in the same matmul N-cycles (M dim is free).
  - PSUM budget (8 banks): sts [128,2x512] x2 bufs = 4, x_ps 2x[128,512]
    = 2, proj/outproj shared pool = 2.
"""

import numpy as np

import concourse.bass as bass
import concourse.tile as tile
from concourse import bacc, mybir
from concourse.bass_utils import run_bass_kernel_spmd

F32 = mybir.dt.float32
F16 = mybir.dt.float16
EXP = mybir.ActivationFunctionType.Exp

N = 2048          # rows (Nq == Nk)
C = 1024          # model dim
HC = 512          # per-core channels (8 heads x 64)
NH = 8            # heads per core
HD = 64           # head dim
KT_TILES = C // 128   # 8 k-tiles over model dim
RCHUNK = 512          # row-chunk for projections
NJT = N // 128        # 16 kj tiles
NQQ = N // 512        # 4 query chunks
SCALE = 0.125         # HEAD_DIM ** -0.5

_CACHE = {}


def _build():
    nc = bacc.Bacc("TRN2", target_bir_lowering=False, debug=False)

    xqT = nc.dram_tensor("xqT", [C, N], F16, kind="ExternalInput")
    xkT = nc.dram_tensor("xkT", [C, N], F16, kind="ExternalInput")
    xvT = nc.dram_tensor("xvT", [C, N], F16, kind="ExternalInput")
    wq = nc.dram_tensor("wq", [C, HC], F16, kind="ExternalInput")
    wk = nc.dram_tensor("wk", [C, HC], F16, kind="ExternalInput")
    wv = nc.dram_tensor("wv", [C, HC], F16, kind="ExternalInput")
    wo = nc.dram_tensor("wo", [HC, C], F16, kind="ExternalInput")
    bq = nc.dram_tensor("bq", [HC], F32, kind="ExternalInput")
    bk = nc.dram_tensor("bk", [HC], F32, kind="ExternalInput")
    bv = nc.dram_tensor("bv", [HC], F32, kind="ExternalInput")
    y = nc.dram_tensor("y", [N, C], F32, kind="ExternalOutput")

    with tile.TileContext(nc) as tc:
        with (
            tc.tile_pool(name="persist", bufs=1) as pp,
            tc.tile_pool(name="xin", bufs=1) as xp,
            tc.tile_pool(name="wts", bufs=1) as wp,
            tc.tile_pool(name="qkt", bufs=2) as qkp,
            tc.tile_pool(name="pt", bufs=2) as ptp,
            tc.tile_pool(name="nrm", bufs=2) as rbp,
            tc.tile_pool(name="yo", bufs=2) as yop,
            tc.tile_pool(name="mm", bufs=2, space="PSUM") as pps,
            tc.tile_pool(name="sts", bufs=2, space="PSUM") as stp,
            tc.tile_pool(name="xps", bufs=1, space="PSUM") as xpp,
        ):
            # ---- biases / constants ----
            bq_sb = pp.tile([128, 4], F32)
            nc.sync.dma_start(bq_sb[:], bq.rearrange("(t p) -> p t", p=128))
            bk_sb = pp.tile([128, 4], F32)
            nc.sync.dma_start(bk_sb[:], bk.rearrange("(t p) -> p t", p=128))
            bv_sb = pp.tile([1, HC], F32)
            nc.sync.dma_start(bv_sb[:], bv.rearrange("(o c) -> o c", o=1))
            bv_bc = pp.tile([128, HC], F32)
            nc.gpsimd.partition_broadcast(bv_bc[:], bv_sb[0:1, :])

            # persistent activations (QT/KT are per-pair, double-buffered)
            V = pp.tile([128, NJT, 4 * 192], F16)  # per kj, per pair [v_e|ones|v_o]
            xT = pp.tile([128, 4, N], F16)    # attention out [pair-ch, pair, qpos]
            QTs = [qkp.tile([128, N], F16, tag="QT", name=f"qt{p}") for p in range(4)]
            KTs = [qkp.tile([128, N], F16, tag="KT", name=f"kt{p}") for p in range(4)]

            # resident inputs [128, kt, rows]
            xq_sb = xp.tile([128, KT_TILES, N], F16)
            xk_sb = xp.tile([128, KT_TILES, N], F16)
            xv_sb = xp.tile([128, KT_TILES, N], F16)
            for dst, src in ((xk_sb, xkT), (xq_sb, xqT), (xv_sb, xvT)):
                r = src.rearrange("(t p) r -> p t r", p=128)
                for rc in range(4):
                    sl = slice(rc * RCHUNK, (rc + 1) * RCHUNK)
                    nc.sync.dma_start(dst[:, :, sl], r[:, :, sl])

            # weights
            wq_sb = wp.tile([128, KT_TILES, HC], F16, tag="wqo", name="wq")
            nc.sync.dma_start(wq_sb[:], wq.rearrange("(t p) n -> p t n", p=128))
            wk_sb = wp.tile([128, KT_TILES, HC], F16, tag="wk", name="wk")
            nc.sync.dma_start(wk_sb[:], wk.rearrange("(t p) n -> p t n", p=128))
            wv_sb = wp.tile([128, KT_TILES, HC], F16, tag="wv", name="wv")
            nc.sync.dma_start(wv_sb[:], wv.rearrange("(t p) n -> p t n", p=128))

            # ones columns of V: pair p cols 192p+64 .. 192p+128
            for pr in range(4):
                nc.vector.memset(V[:, :, 192 * pr + 64:192 * pr + 128], 1.0)

            # preload the exp ACT table so it doesn't stall attention entry
            exp_dump = pp.tile([1, 32], F32)
            nc.scalar.activation(exp_dump[:], bv_bc[0:1, 0:32], EXP, scale=0.0)

            def proj_qk(pair, which):
                """Project q or k for one pair into QT/KT [:, pair, :].

                Out partitions = the pair's 128 channels; bias-add + fp16
                cast on DVE (tensor_scalar_add with per-partition bias AP).
                """
                w_sb, x_sb, b_sb, dstT = (
                    (wq_sb, xq_sb, bq_sb, QTs[pair]) if which == "q"
                    else (wk_sb, xk_sb, bk_sb, KTs[pair])
                )
                for rc in range(4):
                    rsl = slice(rc * RCHUNK, (rc + 1) * RCHUNK)
                    ps = pps.tile([128, RCHUNK], F32, tag="mm",
                                  name=f"ps_{which}{pair}_{rc}")
                    for k in range(KT_TILES):
                        nc.tensor.matmul(
                            ps[:],
                            w_sb[:, k, pair * 128:(pair + 1) * 128],
                            x_sb[:, k, rsl],
                            start=(k == 0),
                            stop=(k == KT_TILES - 1),
                        )
                    nc.vector.tensor_scalar_add(
                        dstT[:, rsl], ps[:], b_sb[:, pair:pair + 1]
                    )

            def proj_v_all():
                """Project v for ALL pairs (N=512 across the 4 pairs' heads).

                Out partitions = kpos rows; scatter heads into the
                [v_e|ones|v_o] packing with bias on DVE.
                """
                for rc in range(4):
                    for rt in range(RCHUNK // 128):
                        kj = rc * (RCHUNK // 128) + rt
                        rsl = slice(rc * RCHUNK + rt * 128,
                                    rc * RCHUNK + (rt + 1) * 128)
                        ps = pps.tile([128, HC], F32, tag="mm", name=f"ps_v{kj}")
                        for k in range(KT_TILES):
                            nc.tensor.matmul(
                                ps[:],
                                xv_sb[:, k, rsl],
                                wv_sb[:, k, :],
                                start=(k == 0),
                                stop=(k == KT_TILES - 1),
                            )
                        ps_h = ps[:].rearrange("p (h d) -> p h d", h=NH)
                        bv_h = bv_bc[:].rearrange("p (h d) -> p h d", h=NH)
                        v_pairs = V[:, kj, :].rearrange("p (pr x) -> p pr x", pr=4)
                        nc.vector.tensor_add(
                            v_pairs[:, :, 0:64], ps_h[:, 0::2, :], bv_h[:, 0::2, :]
                        )
                        nc.vector.tensor_add(
                            v_pairs[:, :, 128:192], ps_h[:, 1::2, :], bv_h[:, 1::2, :]
                        )

            def attn_qq(pair, qq):
                """One 512-query chunk of attention for one head pair."""
                qsl = slice(qq * 512, (qq + 1) * 512)
                x_ps = [
                    xpp.tile([128, 512], F32, tag=f"x{i}", name=f"x_{pair}_{qq}_{i}")
                    for i in range(2)
                ]
                for kj in range(NJT):
                    ksl = slice(kj * 128, (kj + 1) * 128)
                    sts = stp.tile([128, 2, 512], F32, tag="sts",
                                   name=f"st_{pair}_{qq}_{kj}")
                    # row-tiled QK: head0 on rows 0:64, head1 on rows 64:128,
                    # concurrent in the PE array (tile_position auto-derived
                    # from base_partition).
                    for i in range(2):
                        hp = slice(64 * i, 64 * i + 64)
                        nc.tensor.matmul(
                            sts[:, i, :],
                            KTs[pair][hp, ksl],
                            QTs[pair][hp, qsl],
                            start=True,
                            stop=True,
                        )
                    pt = ptp.tile([128, 2, 512], F16, tag="pt",
                                  name=f"pt_{pair}_{qq}_{kj}")
                    nc.scalar.activation(pt[:], sts[:], EXP, scale=SCALE)
                    for i in range(2):
                        Vh = V[:, kj, 192 * pair + 64 * i:192 * pair + 64 * i + 128]
                        nc.tensor.matmul(
                            x_ps[i][:],
                            Vh,
                            pt[:, i, :],
                            start=(kj == 0),
                            stop=(kj == NJT - 1),
                        )
                # normalize on DVE: x_ps rows are [x|den] (i=0) / [den|x] (i=1)
                for i in range(2):
                    xrow, srow = (0, 64) if i == 0 else (64, 0)
                    s_sb = rbp.tile([64, 512], F32, tag="ssb",
                                    name=f"s_{pair}_{qq}_{i}")
                    nc.vector.tensor_copy(s_sb[:], x_ps[i][srow:srow + 64, :])
                    rbc = rbp.tile([64, 512], F32, tag="rbc",
                                   name=f"r_{pair}_{qq}_{i}")
                    nc.vector.reciprocal_approx_fast(rbc[:], s_sb[:])
                    nc.vector.tensor_mul(
                        xT[64 * i:64 * i + 64, pair, qsl],
                        x_ps[i][xrow:xrow + 64, :],
                        rbc[:],
                    )

            wo_holder = {}

            def load_wo():
                wo_sb = wp.tile([128, 4, C], F16, tag="wqo", name="wo")
                nc.sync.dma_start(wo_sb[:], wo.rearrange("(t p) o -> p t o", p=128))
                wo_holder["wo"] = wo_sb

            def outproj_it(it):
                wo_sb = wo_holder["wo"]
                """One 128-row tile of the output projection."""
                ysb = yop.tile([128, C], F32, tag="ysb", name=f"y_{it}")
                for oc in range(2):
                    yps = pps.tile([128, 512], F32, tag="mm", name=f"yp_{it}_{oc}")
                    for ct in range(4):
                        nc.tensor.matmul(
                            yps[:],
                            xT[:, ct, it * 128:(it + 1) * 128],
                            wo_sb[:, ct, oc * 512:(oc + 1) * 512],
                            start=(ct == 0),
                            stop=(ct == 3),
                        )
                    nc.vector.tensor_copy(ysb[:, oc * 512:(oc + 1) * 512], yps[:])
                nc.sync.dma_start(y[it * 128:(it + 1) * 128, :], ysb[:])

            # ---- emission schedule ----
            # prologue: K/Q proj for pair 0, then V for everyone (the first
            # QK only needs K0/Q0; V tiles land before their AVs need them).
            with nc.named_scope("proj_p0"):
                proj_qk(0, "k")
                proj_qk(0, "q")
            with nc.named_scope("proj_v"):
                proj_v_all()

            for pair in range(4):
                with nc.named_scope(f"attn_p{pair}"):
                    for qq in range(NQQ):
                        attn_qq(pair, qq)
                        # overlap: next pair's projections ride the ACT-bound
                        # attention loop, a quarter per qq chunk.
                        if pair < 3:
                            with nc.named_scope(f"proj_p{pair + 1}"):
                                if qq < 2:
                                    proj_qk(pair + 1, "k" if qq == 0 else "q")
                                if pair == 2 and qq == 2:
                                    load_wo()
                        else:
                            # last pair: outproj tiles for finished q rows
                            with nc.named_scope("outproj"):
                                for it in range(4 * qq, 4 * qq + 4):
                                    outproj_it(it)

    nc.finalize()
    return nc


def _get_nc():
    if "nc" not in _CACHE:
        _CACHE["nc"] = _build()
    return _CACHE["nc"]


def _make_in_maps(query, key, value, Wq, bq, Wk, bk, Wv, bv, Wo):
    f = np.float32
    in_maps = []
    for core in range(8):
        b, hg = divmod(core, 2)
        sl = slice(hg * HC, (hg + 1) * HC)
        in_maps.append({
            "xqT": np.ascontiguousarray(np.asarray(query[b], f).T.astype(np.float16)),
            "xkT": np.ascontiguousarray(np.asarray(key[b], f).T.astype(np.float16)),
            "xvT": np.ascontiguousarray(np.asarray(value[b], f).T.astype(np.float16)),
            "wq": np.ascontiguousarray(np.asarray(Wq, f)[:, sl].astype(np.float16)),
            "wk": np.ascontiguousarray(np.asarray(Wk, f)[:, sl].astype(np.float16)),
            "wv": np.ascontiguousarray(np.asarray(Wv, f)[:, sl].astype(np.float16)),
            "wo": np.ascontiguousarray(np.asarray(Wo, f)[sl, :].astype(np.float16)),
            "bq": np.ascontiguousarray(np.asarray(bq, f)[sl]),
            "bk": np.ascontiguousarray(np.asarray(bk, f)[sl]),
            "bv": np.ascontiguousarray(np.asarray(bv, f)[sl]),
        })
    return in_maps


def _run(inputs, trace=False, **kwargs):
    nc = _get_nc()
    in_maps = _make_in_maps(
        inputs["query"], inputs["key"], inputs["value"],
        inputs["Wq"], inputs["bq"], inputs["Wk"], inputs["bk"],
        inputs["Wv"], inputs["bv"], inputs["Wo"],
    )
    res = run_bass_kernel_spmd(nc, in_maps, core_ids=list(range(8)), trace=trace, **kwargs)
    bo = np.asarray(inputs["bo"], np.float32)
    out = np.empty((4, N, C), np.float32)
    for b in range(4):
        out[b] = res.results[2 * b]["y"] + res.results[2 * b + 1]["y"] + bo
    return out, res


def kernel(**inputs) -> np.ndarray:
    out, _ = _run(inputs, trace=False)
    return out


# revision 9
# speedup vs baseline: 1.1778x; 1.1778x over previous
"""CrossAttention Trainium2 kernel (8 NeuronCores, Bass/Tile).

Problem: B=4, Nq=Nk=2048, DIM=1024, HEADS=16, HEAD_DIM=64, fp32.
  q = query @ Wq + bq ; k = key @ Wk + bk ; v = value @ Wv + bv
  attn = softmax(q k^T / 8) ; x = attn v ; out = x @ Wo + bo

Sharding: 8 cores = 4 batches x 2 head-groups (8 heads, 512 channels each).
Each core computes y_partial[b] = (attn-out restricted to its 512 channels)
@ Wo_rows; host sums the two partials per batch and adds bo.

v2 design (ACT-bound software pipeline):
  - The softmax exp is the hard floor: 8 heads x 2048^2 scores = 33.5M
    ACT-lane-cycles ~ 270us busy. Everything else hides under it:
    * All projections + outproj interleave with the attention loop
      (per-pair: K/Q proj of pair p+1 emitted inside pair p's attention).
    * ACT does EXP only; every copy/bias-add/normalize is DVE.
  - QK is row-tiled: the two heads of a pair contract K=64 each on row
    groups (0,0)/(64,0) concurrently -> no zero-padding waste.
  - AV keeps the [v_even|ones|v_odd] packing: the ones columns produce the
    softmax denominator in the same matmul N-cycles (M dim is free).
  - PSUM budget (8 banks): sts [128,2x512] x2 bufs = 4, x_ps 2x[128,512]
    = 2, proj/outproj shared pool = 2.
"""

import numpy as np

import concourse.bass as bass
import concourse.tile as tile
from concourse import bacc, mybir
from concourse.bass_utils import run_bass_kernel_spmd

F32 = mybir.dt.float32
F16 = mybir.dt.float16
EXP = mybir.ActivationFunctionType.Exp

N = 2048          # rows (Nq == Nk)
C = 1024          # model dim
HC = 512          # per-core channels (8 heads x 64)
NH = 8            # heads per core
HD = 64           # head dim
KT_TILES = C // 128   # 8 k-tiles over model dim
RCHUNK = 512          # row-chunk for projections
NJT = N // 128        # 16 kj tiles
NQQ = N // 512        # 4 query chunks
SCALE = 0.125         # HEAD_DIM ** -0.5

_CACHE = {}


def _build():
    nc = bacc.Bacc("TRN2", target_bir_lowering=False, debug=False)

    xqT = nc.dram_tensor("xqT", [C, N], F16, kind="ExternalInput")
    xkT = nc.dram_tensor("xkT", [C, N], F16, kind="ExternalInput")
    xvT = nc.dram_tensor("xvT", [C, N], F16, kind="ExternalInput")
    wq = nc.dram_tensor("wq", [C, HC], F16, kind="ExternalInput")
    wk = nc.dram_tensor("wk", [C, HC], F16, kind="ExternalInput")
    wv = nc.dram_tensor("wv", [C, HC], F16, kind="ExternalInput")
    wo = nc.dram_tensor("wo", [HC, C], F16, kind="ExternalInput")
    bq = nc.dram_tensor("bq", [HC], F32, kind="ExternalInput")
    bk = nc.dram_tensor("bk", [HC], F32, kind="ExternalInput")
    bv = nc.dram_tensor("bv", [HC], F32, kind="ExternalInput")
    y = nc.dram_tensor("y", [N, C], F16, kind="ExternalOutput")

    with tile.TileContext(nc) as tc:
        with (
            tc.tile_pool(name="persist", bufs=1) as pp,
            tc.tile_pool(name="xin", bufs=1) as xp,
            tc.tile_pool(name="wts", bufs=1) as wp,
            tc.tile_pool(name="qkt", bufs=2) as qkp,
            tc.tile_pool(name="pt", bufs=3) as ptp,
            tc.tile_pool(name="nrm", bufs=2) as rbp,
            tc.tile_pool(name="yo", bufs=2) as yop,
            tc.tile_pool(name="mm", bufs=2, space="PSUM") as pps,
            tc.tile_pool(name="sts", bufs=2, space="PSUM") as stp,
            tc.tile_pool(name="xps", bufs=1, space="PSUM") as xpp,
        ):
            # ---- biases / constants ----
            bq_sb = pp.tile([128, 4], F32)
            nc.sync.dma_start(bq_sb[:], bq.rearrange("(t p) -> p t", p=128))
            bk_sb = pp.tile([128, 4], F32)
            nc.sync.dma_start(bk_sb[:], bk.rearrange("(t p) -> p t", p=128))
            bv_sb = pp.tile([1, HC], F32)
            nc.sync.dma_start(bv_sb[:], bv.rearrange("(o c) -> o c", o=1))
            bv_bc = pp.tile([128, HC], F32)
            nc.gpsimd.partition_broadcast(bv_bc[:], bv_sb[0:1, :])

            # persistent activations (QT/KT are per-pair, double-buffered)
            V = pp.tile([128, NJT, 4 * 192], F16)  # per kj, per pair [v_e|ones|v_o]
            xT = pp.tile([128, 4, N], F16)    # attention out [pair-ch, pair, qpos]
            QTs = [qkp.tile([128, N], F16, tag="QT", name=f"qt{p}") for p in range(4)]
            KTs = [qkp.tile([128, N], F16, tag="KT", name=f"kt{p}") for p in range(4)]

            # resident inputs [128, kt, rows]; DMAs emitted in first-use
            # order: wk,xk0,wq,xq0 feed the first QK; wv+xv feed V-proj
            # which streams into pair0-qq0's AV; the rest follow.
            xq_sb = xp.tile([128, KT_TILES, N], F16)
            xk_sb = xp.tile([128, KT_TILES, N], F16)
            xv_sb = xp.tile([128, KT_TILES, N], F16)
            xq_r = xqT.rearrange("(t p) r -> p t r", p=128)
            xk_r = xkT.rearrange("(t p) r -> p t r", p=128)
            xv_r = xvT.rearrange("(t p) r -> p t r", p=128)

            def dma_chunk(dst, r, rc):
                sl = slice(rc * RCHUNK, (rc + 1) * RCHUNK)
                nc.sync.dma_start(dst[:, :, sl], r[:, :, sl])

            wk_sb = wp.tile([128, KT_TILES, HC], F16, tag="wk", name="wk")
            nc.sync.dma_start(wk_sb[:], wk.rearrange("(t p) n -> p t n", p=128))
            dma_chunk(xk_sb, xk_r, 0)
            wq_sb = wp.tile([128, KT_TILES, HC], F16, tag="wqo", name="wq")
            nc.sync.dma_start(wq_sb[:], wq.rearrange("(t p) n -> p t n", p=128))
            dma_chunk(xq_sb, xq_r, 0)
            wv_sb = wp.tile([128, KT_TILES, HC], F16, tag="wv", name="wv")
            nc.sync.dma_start(wv_sb[:], wv.rearrange("(t p) n -> p t n", p=128))
            dma_chunk(xv_sb, xv_r, 0)
            dma_chunk(xv_sb, xv_r, 1)
            dma_chunk(xk_sb, xk_r, 1)
            dma_chunk(xv_sb, xv_r, 2)
            dma_chunk(xk_sb, xk_r, 2)
            dma_chunk(xv_sb, xv_r, 3)
            dma_chunk(xk_sb, xk_r, 3)
            for rc in range(1, 4):
                dma_chunk(xq_sb, xq_r, rc)

            # ones columns of V: pair p cols 192p+64 .. 192p+128
            for pr in range(4):
                nc.vector.memset(V[:, :, 192 * pr + 64:192 * pr + 128], 1.0)

            # preload the exp ACT table so it doesn't stall attention entry
            exp_dump = pp.tile([1, 32], F32)
            nc.scalar.activation(exp_dump[:], bv_bc[0:1, 0:32], EXP, scale=0.0)

            ps_holder = {}

            def proj_qk_half(pair, which, rc, half):
                """Half of one 512-row chunk of a q/k projection (4 matmuls).

                The 8-matmul accumulation chain for chunk rc is split in two
                ~0.9us bursts so interleaved emission never blocks the next
                QK long enough to starve ACT. half=1 finishes the chain and
                does the DVE bias-add into QTs/KTs.
                """
                w_sb, x_sb, b_sb, dstT = (
                    (wq_sb, xq_sb, bq_sb, QTs[pair]) if which == "q"
                    else (wk_sb, xk_sb, bk_sb, KTs[pair])
                )
                rsl = slice(rc * RCHUNK, (rc + 1) * RCHUNK)
                key = (pair, which, rc)
                if half == 0:
                    ps_holder[key] = pps.tile([128, RCHUNK], F32, tag="mm",
                                              name=f"ps_{which}{pair}_{rc}")
                ps = ps_holder[key]
                for k in range(4 * half, 4 * half + 4):
                    nc.tensor.matmul(
                        ps[:],
                        w_sb[:, k, pair * 128:(pair + 1) * 128],
                        x_sb[:, k, rsl],
                        start=(k == 0),
                        stop=(k == KT_TILES - 1),
                    )
                if half == 1:
                    nc.vector.tensor_scalar_add(
                        dstT[:, rsl], ps[:], b_sb[:, pair:pair + 1]
                    )

            def proj_v_row(kj):
                """Project v rows kj*128..+128 for ALL pairs (one kj tile)."""
                rsl = slice(kj * 128, (kj + 1) * 128)
                ps = pps.tile([128, HC], F32, tag="mm", name=f"ps_v{kj}")
                for k in range(KT_TILES):
                    nc.tensor.matmul(
                        ps[:],
                        xv_sb[:, k, rsl],
                        wv_sb[:, k, :],
                        start=(k == 0),
                        stop=(k == KT_TILES - 1),
                    )
                ps_h = ps[:].rearrange("p (h d) -> p h d", h=NH)
                bv_h = bv_bc[:].rearrange("p (h d) -> p h d", h=NH)
                v_pairs = V[:, kj, :].rearrange("p (pr x) -> p pr x", pr=4)
                nc.vector.tensor_add(
                    v_pairs[:, :, 0:64], ps_h[:, 0::2, :], bv_h[:, 0::2, :]
                )
                nc.vector.tensor_add(
                    v_pairs[:, :, 128:192], ps_h[:, 1::2, :], bv_h[:, 1::2, :]
                )

            wo_holder = {}

            def load_wo():
                wo_sb = wp.tile([128, 4, C], F16, tag="wqo", name="wo")
                nc.sync.dma_start(wo_sb[:], wo.rearrange("(t p) o -> p t o", p=128))
                wo_holder["wo"] = wo_sb

            yps_holder = {}

            def outproj_half(it, half):
                """Half of one 128-row output-projection tile (4 matmuls).

                ct-outer/oc-inner so each xT ldweights serves two matmuls.
                """
                if half == 0:
                    yps_holder[it] = [
                        pps.tile([128, 512], F32, tag="mm", name=f"yp_{it}_{oc}")
                        for oc in range(2)
                    ]
                yps = yps_holder[it]
                wo_sb = wo_holder["wo"]
                for ct in (2 * half, 2 * half + 1):
                    for oc in range(2):
                        nc.tensor.matmul(
                            yps[oc][:],
                            xT[:, ct, it * 128:(it + 1) * 128],
                            wo_sb[:, ct, oc * 512:(oc + 1) * 512],
                            start=(ct == 0),
                            stop=(ct == 3),
                        )
                if half == 1:
                    ysb = yop.tile([128, C], F16, tag="ysb", name=f"y_{it}")
                    for oc in range(2):
                        nc.vector.tensor_copy(
                            ysb[:, oc * 512:(oc + 1) * 512], yps[oc][:]
                        )
                    nc.sync.dma_start(y[it * 128:(it + 1) * 128, :], ysb[:])

            # ---- interleave hook table ----
            # hooks[(pair, qq, kj)] -> list of closures emitted right after
            # that kj step's instructions. Keep bursts <= ~1.7us.
            hooks = {}

            def add_hook(pair, qq, kj, fn):
                hooks.setdefault((pair, qq, kj), []).append(fn)

            def H(fn, *a):
                return lambda: fn(*a)

            # pair0 qq0: V rows stream in just ahead of their AV; K chunks
            # land before the QK that reads them.
            for kj in range(14):
                add_hook(0, 0, kj, H(proj_v_row, kj + 2))
            for rc in range(1, 4):
                base = 4 * rc - 3
                add_hook(0, 0, base, H(proj_qk_half, 0, "k", rc, 0))
                add_hook(0, 0, base + 1, H(proj_qk_half, 0, "k", rc, 1))
            add_hook(0, 0, 12, H(proj_qk_half, 0, "q", 1, 0))
            add_hook(0, 0, 13, H(proj_qk_half, 0, "q", 1, 1))
            # Q chunk rc lands during qq rc-1 (pair3: all inside qq0 so wq
            # is released before the wo load that outproj waits on)
            for p in range(3):
                for rc in range(2, 4):
                    add_hook(p, rc - 1, 4, H(proj_qk_half, p, "q", rc, 0))
                    add_hook(p, rc - 1, 5, H(proj_qk_half, p, "q", rc, 1))
            add_hook(3, 0, 6, H(proj_qk_half, 3, "q", 2, 0))
            add_hook(3, 0, 7, H(proj_qk_half, 3, "q", 2, 1))
            add_hook(3, 0, 10, H(proj_qk_half, 3, "q", 3, 0))
            add_hook(3, 0, 11, H(proj_qk_half, 3, "q", 3, 1))
            for p in range(1, 4):
                add_hook(p, 0, 2, H(proj_qk_half, p, "q", 1, 0))
                add_hook(p, 0, 3, H(proj_qk_half, p, "q", 1, 1))
            # next pair's K during qq2/qq3; its Q rc0 at qq3 tail
            for p in range(3):
                for rc in range(4):
                    qq = 2 + rc // 2
                    base = 1 + 4 * (rc % 2)
                    add_hook(p, qq, base, H(proj_qk_half, p + 1, "k", rc, 0))
                    add_hook(p, qq, base + 1, H(proj_qk_half, p + 1, "k", rc, 1))
                add_hook(p, 3, 9, H(proj_qk_half, p + 1, "q", 0, 0))
                add_hook(p, 3, 10, H(proj_qk_half, p + 1, "q", 0, 1))
            add_hook(2, 3, 13, H(load_wo))
            # outproj: rows of qq land during pair3's qq+1 (qq3 in the tail)
            for qq in range(1, 4):
                for j in range(4):
                    add_hook(3, qq, 3 * j + 1, H(outproj_half, 4 * (qq - 1) + j, 0))
                    add_hook(3, qq, 3 * j + 2, H(outproj_half, 4 * (qq - 1) + j, 1))

            # ---- flat attention pipeline ----
            # AV lags one kj step behind QK/EXP; x_ps normalize is emitted
            # after the AV flush that crosses into the next chunk, so the
            # ACT stream never waits on PE/DVE epilogues.
            pend_av = None      # (pair, x_ps, pt, kj)
            pend_norm = None    # (pair, qq, x_ps)

            def flush_av():
                nonlocal pend_av
                if pend_av is None:
                    return
                pair, x_ps, pt, kj = pend_av
                for i in range(2):
                    Vh = V[:, kj, 192 * pair + 64 * i:192 * pair + 64 * i + 128]
                    nc.tensor.matmul(
                        x_ps[i][:],
                        Vh,
                        pt[:, i, :],
                        start=(kj == 0),
                        stop=(kj == NJT - 1),
                    )
                pend_av = None

            def flush_norm():
                nonlocal pend_norm
                if pend_norm is None:
                    return
                pair, qq, x_ps = pend_norm
                qsl = slice(qq * 512, (qq + 1) * 512)
                for i in range(2):
                    xrow, srow = (0, 64) if i == 0 else (64, 0)
                    s_sb = rbp.tile([64, 512], F32, tag="ssb",
                                    name=f"s_{pair}_{qq}_{i}")
                    nc.vector.tensor_copy(s_sb[:], x_ps[i][srow:srow + 64, :])
                    rbc = rbp.tile([64, 512], F32, tag="rbc",
                                   name=f"r_{pair}_{qq}_{i}")
                    nc.vector.reciprocal_approx_fast(rbc[:], s_sb[:])
                    nc.vector.tensor_mul(
                        xT[64 * i:64 * i + 64, pair, qsl],
                        x_ps[i][xrow:xrow + 64, :],
                        rbc[:],
                    )
                pend_norm = None

            # prologue: first chunks of K0/Q0 and the first two V rows
            with nc.named_scope("prologue"):
                proj_qk_half(0, "k", 0, 0)
                proj_qk_half(0, "k", 0, 1)
                proj_qk_half(0, "q", 0, 0)
                proj_qk_half(0, "q", 0, 1)
                proj_v_row(0)
                proj_v_row(1)

            for pair in range(4):
                with nc.named_scope(f"attn_p{pair}"):
                    for qq in range(NQQ):
                        qsl = slice(qq * 512, (qq + 1) * 512)
                        x_ps = [
                            xpp.tile([128, 512], F32, tag=f"x{i}",
                                     name=f"x_{pair}_{qq}_{i}")
                            for i in range(2)
                        ]
                        for kj in range(NJT):
                            ksl = slice(kj * 128, (kj + 1) * 128)
                            sts = stp.tile([128, 2, 512], F32, tag="sts",
                                           name=f"st_{pair}_{qq}_{kj}")
                            for i in range(2):
                                hp = slice(64 * i, 64 * i + 64)
                                nc.tensor.matmul(
                                    sts[:, i, :],
                                    KTs[pair][hp, ksl],
                                    QTs[pair][hp, qsl],
                                    start=True,
                                    stop=True,
                                )
                            flush_av()
                            flush_norm()
                            pt = ptp.tile([128, 2, 512], F16, tag="pt",
                                          name=f"pt_{pair}_{qq}_{kj}")
                            nc.scalar.activation(pt[:], sts[:], EXP, scale=SCALE)
                            pend_av = (pair, x_ps, pt, kj)
                            for fn in hooks.get((pair, qq, kj), ()):
                                fn()
                        pend_norm = (pair, qq, x_ps)

            # tail: last AV + normalize + final outproj tiles
            with nc.named_scope("tail"):
                flush_av()
                flush_norm()
                for j in range(4):
                    outproj_half(12 + j, 0)
                    outproj_half(12 + j, 1)

    nc.finalize()
    return nc


def _get_nc():
    if "nc" not in _CACHE:
        _CACHE["nc"] = _build()
    return _CACHE["nc"]


def _make_in_maps(query, key, value, Wq, bq, Wk, bk, Wv, bv, Wo):
    f = np.float32
    in_maps = []
    for core in range(8):
        b, hg = divmod(core, 2)
        sl = slice(hg * HC, (hg + 1) * HC)
        in_maps.append({
            "xqT": np.ascontiguousarray(np.asarray(query[b], f).T.astype(np.float16)),
            "xkT": np.ascontiguousarray(np.asarray(key[b], f).T.astype(np.float16)),
            "xvT": np.ascontiguousarray(np.asarray(value[b], f).T.astype(np.float16)),
            "wq": np.ascontiguousarray(np.asarray(Wq, f)[:, sl].astype(np.float16)),
            "wk": np.ascontiguousarray(np.asarray(Wk, f)[:, sl].astype(np.float16)),
            "wv": np.ascontiguousarray(np.asarray(Wv, f)[:, sl].astype(np.float16)),
            "wo": np.ascontiguousarray(np.asarray(Wo, f)[sl, :].astype(np.float16)),
            "bq": np.ascontiguousarray(np.asarray(bq, f)[sl]),
            "bk": np.ascontiguousarray(np.asarray(bk, f)[sl]),
            "bv": np.ascontiguousarray(np.asarray(bv, f)[sl]),
        })
    return in_maps


def _run(inputs, trace=False, **kwargs):
    nc = _get_nc()
    in_maps = _make_in_maps(
        inputs["query"], inputs["key"], inputs["value"],
        inputs["Wq"], inputs["bq"], inputs["Wk"], inputs["bk"],
        inputs["Wv"], inputs["bv"], inputs["Wo"],
    )
    res = run_bass_kernel_spmd(nc, in_maps, core_ids=list(range(8)), trace=trace, **kwargs)
    bo = np.asarray(inputs["bo"], np.float32)
    out = np.empty((4, N, C), np.float32)
    for b in range(4):
        out[b] = (res.results[2 * b]["y"].astype(np.float32)
                  + res.results[2 * b + 1]["y"].astype(np.float32) + bo)
    return out, res


def kernel(**inputs) -> np.ndarray:
    out, _ = _run(inputs, trace=False)
    return out
